# revision 1
# baseline (speedup 1.0000x reference)
"""Fused bmm + residual kernel for Trainium2 (8 NeuronCores, data-parallel).

out[n,c,p] = x[n,c,p] + alpha * sum_q attn[n,p,q] * D[n,q,c]
  N=2048, C=512, H=W=7 (HW=49)

Sharding: batch N across 8 cores (256 each), no collectives.

Scheme (tolerance is 2e-2; measured max rel err ~1.1e-2):
 - Host pre-packs all tensors to device layouts, so every DMA is a
   contiguous partition-major block with multi-KB descriptors.
 - attn is transposed on the host (alpha folded in); no device transposes.
 - Pair packing with K=98, gap-free: even batch at partitions 0:49, odd
   at 49:98 (matmul operand partition base MUST be 0 here: bases 32/64
   pass bass asserts but base-64 aborts on this runtime).
 - rhs [98, (2, 49)] is block-diagonal, off-diag zeroed once per ring
   tile; lhsT = D pair [98, 512] in c-chunks of 128; 4 MMs + 1 add /pair.
 - D loads as ONE 98-row DMA per group: HWDGE spreads a DMA over
   min(16, ceil(rows/7)) SDMA engines by relative row, so 98 rows ride
   14 engines (49-row DMAs would pile onto 7).  Per-engine ~21.5 GB/s is
   the roofline; busiest-engine bytes set the kernel time.
 - x loads in fp8-e4m3: its rounding error is additive (~0.28 abs vs the
   ~0.51 abs tolerance budget) and does not pass through the contraction,
   so fp8 is safe for x but NOT for D/attn.  D/attn/out use bf16.
 - PSUM y [128, 4, 2, (2, 49)] fp32 covers TWO pairs; one DVE add per 2
   pairs (FD=784 amortizes the ~120-cycle DVE op startup).
 - Load/store DMAs alternate between the two HWDGE rings per group.
 - Variable group sizes: small warmup groups start the matmuls early,
   small tail groups shorten the drain.
"""
import sys

sys.path.insert(0, "/opt/trn_rl_repo")

import numpy as np
import ml_dtypes

BF16 = ml_dtypes.bfloat16
FP8 = ml_dtypes.float8_e4m3fn

# ---- static problem config (hardcoded per harness contract) ----
N_TOT, C, HW = 2048, 512, 49
N_CORES = 8
NB = N_TOT // N_CORES        # 256 batches per core
NPAIRS = NB // 2             # 128 pairs per core
GMAX = 16                    # max batches per group
GROUPS = [4, 4, 8] + [16] * 14 + [8, 4, 4]   # batches per group, sum=256
assert sum(GROUPS) == NB and all(g % 4 == 0 for g in GROUPS)
NAT = 4                      # rhs (attn^T) ring size

_cached = {}


def _build_bass():
    import concourse.bacc as bacc
    import concourse.mybir as mybir
    from concourse import tile

    bf16 = mybir.dt.bfloat16
    fp8 = mybir.dt.float8e4
    f32 = mybir.dt.float32
    nc = bacc.Bacc(None, target_bir_lowering=False)

    x_d = nc.dram_tensor("xp", [128, NB, 4, HW], fp8, kind="ExternalInput")
    ao_d = nc.dram_tensor("ao", [HW, NPAIRS, HW], bf16, kind="ExternalInput")
    cb_d = nc.dram_tensor("cb", [2 * HW, NPAIRS, C + HW], bf16, kind="ExternalInput")
    o_d = nc.dram_tensor("op", [128, NB, 4, HW], bf16, kind="ExternalOutput")

    with tile.TileContext(nc) as tc:
        with (
            tc.tile_pool(name="atp", bufs=NAT) as atp,
            tc.tile_pool(name="dp", bufs=4) as dp,
            tc.tile_pool(name="xp", bufs=5) as xp,
            tc.tile_pool(name="op", bufs=5) as op,
            tc.tile_pool(name="yp", bufs=4, space="PSUM") as yp,
        ):
            # rhs ring: block-diagonal [98, (2, 49)] per pair; off-diagonal
            # blocks must stay zero, so memset each ring tile once up front.
            at_tiles = []
            for i in range(NAT):
                t = atp.tile([2 * HW, 2, GMAX // 2, HW], bf16, tag="at")
                nc.vector.memset(t, 0.0)
                at_tiles.append(t)

            b0 = 0
            for g, gsz in enumerate(GROUPS):
                npair = gsz // 2
                i0 = b0 // 2
                ld = nc.sync if g % 2 == 0 else nc.scalar
                st = nc.scalar if g % 2 == 0 else nc.sync

                d_t = dp.tile([2 * HW, GMAX // 2, C + HW], bf16, tag="d")
                ld.dma_start(out=d_t[:, 0:npair], in_=cb_d[:, i0:i0 + npair])

                at_t = at_tiles[g % NAT]
                # even diagonal block: base-0 copy on the idle GpSimd engine
                nc.gpsimd.tensor_copy(
                    out=at_t[0:HW, 0, 0:npair, :],
                    in_=d_t[0:HW, 0:npair, C:C + HW],
                )
                ld.dma_start(out=at_t[HW:2 * HW, 1, 0:npair, :],
                             in_=ao_d[:, i0:i0 + npair])

                x_t = xp.tile([128, GMAX, 4, HW], fp8, tag="x")
                st.dma_start(out=x_t[:, 0:gsz], in_=x_d[:, b0:b0 + gsz])

                o_t = op.tile([128, GMAX, 4, HW], bf16, tag="o")

                for ii in range(npair // 2):
                    # y covers two pairs: free dims (j, u, (b, p))
                    y_ps = yp.tile([128, 4, 2, 2 * HW], f32, tag="y")
                    for u in range(2):
                        i = 2 * ii + u
                        for j in range(4):
                            nc.tensor.matmul(
                                out=y_ps[:, j, u, :],
                                lhsT=d_t[0:2 * HW, i, 128 * j:128 * (j + 1)],
                                rhs=at_t[0:2 * HW, :, i, :],
                                start=True,
                                stop=True,
                            )
                    # regroup (j, u, b, p) -> (u, b, j, p) = (n, j, p)
                    y4 = y_ps.rearrange("r j u (b p) -> r (u b) j p", b=2)
                    nc.vector.tensor_add(
                        out=o_t[:, 4 * ii:4 * ii + 4, :, :],
                        in0=y4,
                        in1=x_t[:, 4 * ii:4 * ii + 4, :, :],
                    )

                st.dma_start(out=o_d[:, b0:b0 + gsz], in_=o_t[:, 0:gsz])
                b0 += gsz

    nc.finalize()
    return nc


def _get_nc():
    if "nc" not in _cached:
        _cached["nc"] = _build_bass()
    return _cached["nc"]


def _in_maps(x, attn, D, alpha):
    a0 = np.float32(np.asarray(alpha).reshape(-1)[0])

    # x[n, c, p] -> xp[core, r, n, j, p] with c = 128j + r
    xr = np.asarray(x, dtype=np.float32).reshape(N_CORES, NB, 4, 128, HW)
    xp = np.ascontiguousarray(xr.transpose(0, 3, 1, 2, 4)).astype(FP8)

    # attn[n, p, q] * alpha -> attn_T[n, q, p] -> [core, q, i, p] even/odd
    at = (np.asarray(attn, dtype=np.float32) * a0).transpose(0, 2, 1)
    at = at.reshape(N_CORES, NPAIRS, 2, HW, HW)
    ao = np.ascontiguousarray(at[:, :, 1].transpose(0, 2, 1, 3)).astype(BF16)

    # combined [core, (b,q), i, 561]: cols 0:512 = D rows; even rows carry
    # alpha*A^T_even at cols 512:561; odd rows' attn columns unused (zero)
    dr = np.asarray(D, dtype=np.float32).reshape(N_CORES, NPAIRS, 2, HW, C)
    cb = np.zeros((N_CORES, 2 * HW, NPAIRS, C + HW), dtype=BF16)
    cb[:, 0:HW, :, 0:C] = dr[:, :, 0].transpose(0, 2, 1, 3).astype(BF16)
    cb[:, 0:HW, :, C:] = at[:, :, 0].transpose(0, 2, 1, 3).astype(BF16)
    cb[:, HW:2 * HW, :, 0:C] = dr[:, :, 1].transpose(0, 2, 1, 3).astype(BF16)

    return [
        {"xp": xp[c], "ao": ao[c], "cb": cb[c]}
        for c in range(N_CORES)
    ]


def kernel(x: np.ndarray, attn: np.ndarray, D: np.ndarray, alpha: np.ndarray) -> np.ndarray:
    from concourse import bass_utils

    nc = _get_nc()
    res = bass_utils.run_bass_kernel_spmd(
        nc, _in_maps(x, attn, D, alpha), core_ids=list(range(N_CORES))
    )
    # op[r, n, j, p] -> out[n, c, p] with c = 128j + r
    out = np.stack([res.results[c]["op"] for c in range(N_CORES)])
    out = out.astype(np.float32).transpose(0, 2, 3, 1, 4)
    return np.ascontiguousarray(out).reshape(N_TOT, C, 7, 7)



# revision 7
# speedup vs baseline: 1.3523x; 1.3523x over previous
"""Fused bmm + residual kernel for Trainium2 (8 NeuronCores, data-parallel).

out[n,c,p] = x[n,c,p] + alpha * sum_q attn[n,p,q] * D[n,q,c]
  N=2048, C=512, H=W=7 (HW=49)

Sharding: batch N across 8 cores (256 each), no collectives.

Scheme (tolerance 2e-2; this scheme measures ~1.2e-2):
 - DMA-byte-bound problem: every input rides int8 (1 B/elem).  Host
   quantizes x, attn^T, D each with a per-tensor scale s = max|v|/127.
   int8 beats fp8-e4m3 for gaussian data: max err s/2 instead of 2^-4
   relative on the bulk.
 - The matmul is EXACT: int8 values convert to bf16 (ints <= 127 are
   exact in bf16), PE multiplies exactly (products <= 16129), PSUM fp32
   accumulates 98-term integer sums < 2^24 exactly.
 - All dequant scales collapse into one runtime scalar
   lam = alpha*s_a*s_d/s_x, shipped as a [128,1] fp32 input and applied
   by one DVE scalar_tensor_tensor per 2 pairs:
     o' = y_psum * lam + x_q   (o' is out/s_x; host multiplies by s_x)
 - Pair packing K=98: even batch at partitions 0:49, odd at 49:98
   (matmul operand partition base MUST be 0 on this runtime).
 - cb[98, pair, 610] int8 carries D (cols 0:512) AND the block-diagonal
   attn^T rhs (cols 512:610 = (slot, p), zeros pre-packed on host), so
   ONE act-engine Copy per group converts everything to bf16 exactly.
   No per-group gpsimd copies/memsets and no SBUF ring invariants --
   partition-base rules forbid engine copies at base 49, and gpsimd
   casts measure ~1.2us each anyway.
 - rhs per pair = cb16[:, i, 512:610] viewed as [98, 2, 49]; the zero
   off-diagonal halves come from the host packing.
 - 98-row cb DMAs ride 14 of the 16 SDMA engines (HWDGE spreads by
   relative row, ~7 rows/engine, ~19.7 GB/s/engine); 128-row x/out DMAs
   ride all 16.  Load/store DMAs alternate between two HWDGE rings
   (sync / gpsimd trigger queues).
 - PSUM y [128, (u, b, j, p)] fp32 covers TWO pairs; (u b j) flattens to
   a uniform-stride 3D view because stt requires <=3D operands.
 - Variable group sizes: small warmup groups start the matmuls early,
   small tail groups shorten the drain.
"""
import sys

sys.path.insert(0, "/opt/trn_rl_repo")

import numpy as np
import ml_dtypes

BF16 = ml_dtypes.bfloat16

# ---- static problem config (hardcoded per harness contract) ----
N_TOT, C, HW = 2048, 512, 49
N_CORES = 8
NB = N_TOT // N_CORES        # 256 batches per core
NPAIRS = NB // 2             # 128 pairs per core
CW = C + 2 * HW              # 610 combined columns (D | diag-attn rhs)
GMAX = 16                    # max batches per group
GROUPS = [4, 4, 8] + [16] * 14 + [8, 4, 4]   # batches per group, sum=256
assert sum(GROUPS) == NB and all(g % 4 == 0 for g in GROUPS)

_cached = {}


def _build_bass():
    import concourse.bacc as bacc
    import concourse.mybir as mybir
    from concourse import tile

    bf16 = mybir.dt.bfloat16
    i8 = mybir.dt.int8
    f32 = mybir.dt.float32
    MUL = mybir.AluOpType.mult
    ADD = mybir.AluOpType.add
    nc = bacc.Bacc(None, target_bir_lowering=False)

    x_d = nc.dram_tensor("xp", [128, NB, 4, HW], i8, kind="ExternalInput")
    cb_d = nc.dram_tensor("cb", [2 * HW, NPAIRS, CW], i8, kind="ExternalInput")
    lam_d = nc.dram_tensor("lam", [128, 1], f32, kind="ExternalInput")
    o_d = nc.dram_tensor("op", [128, NB, 4, HW], bf16, kind="ExternalOutput")

    with tile.TileContext(nc) as tc:
        with (
            tc.tile_pool(name="dp", bufs=4) as dp,
            tc.tile_pool(name="d16p", bufs=4) as d16p,
            tc.tile_pool(name="xp", bufs=5) as xp,
            tc.tile_pool(name="op", bufs=5) as op,
            tc.tile_pool(name="lamp", bufs=1) as lamp,
            tc.tile_pool(name="yp", bufs=4, space="PSUM") as yp,
        ):
            lam_t = lamp.tile([128, 1], f32, tag="lam")
            nc.sync.dma_start(out=lam_t, in_=lam_d[:, :])

            b0 = 0
            for g, gsz in enumerate(GROUPS):
                npair = gsz // 2
                i0 = b0 // 2
                ld = nc.sync if g % 2 == 0 else nc.gpsimd
                st = nc.gpsimd if g % 2 == 0 else nc.sync

                d_t = dp.tile([2 * HW, GMAX // 2, CW], i8, tag="d")
                ld.dma_start(out=d_t[:, 0:npair], in_=cb_d[:, i0:i0 + npair])

                # exact int8 -> bf16 dequant of D + diag-attn in one op
                d16 = d16p.tile([2 * HW, GMAX // 2, CW], bf16, tag="d16")
                nc.scalar.copy(out=d16[:, 0:npair], in_=d_t[:, 0:npair])

                x_t = xp.tile([128, GMAX, 4, HW], i8, tag="x")
                st.dma_start(out=x_t[:, 0:gsz], in_=x_d[:, b0:b0 + gsz])

                o_t = op.tile([128, GMAX, 4, HW], bf16, tag="o")

                for ii in range(npair // 2):
                    # y covers two pairs: free dims (u, b, j, p) so that
                    # (u b j) flattens to a uniform-stride 3D view for the
                    # DVE op (stt requires <=3D operands)
                    y_ps = yp.tile([128, 2, 2, 4, HW], f32, tag="y")
                    for u in range(2):
                        i = 2 * ii + u
                        rhs = d16[:, i, C:CW].rearrange(
                            "r (s p) -> r s p", s=2)
                        for j in range(4):
                            nc.tensor.matmul(
                                out=y_ps[:, u, :, j, :],
                                lhsT=d16[0:2 * HW, i, 128 * j:128 * (j + 1)],
                                rhs=rhs,
                                start=True,
                                stop=True,
                            )
                    # o' = y*lam + x_q  (one DVE op per 2 pairs); all three
                    # operands flatten to [128, 16, 49]
                    y3 = y_ps.rearrange("r u b j p -> r (u b j) p")
                    nc.vector.scalar_tensor_tensor(
                        out=o_t[:, 4 * ii:4 * ii + 4, :, :].rearrange(
                            "r n j p -> r (n j) p"),
                        in0=y3,
                        scalar=lam_t[:, 0:1],
                        in1=x_t[:, 4 * ii:4 * ii + 4, :, :].rearrange(
                            "r n j p -> r (n j) p"),
                        op0=MUL,
                        op1=ADD,
                    )

                st.dma_start(out=o_d[:, b0:b0 + gsz], in_=o_t[:, 0:gsz])
                b0 += gsz

    nc.finalize()
    return nc


def _get_nc():
    if "nc" not in _cached:
        _cached["nc"] = _build_bass()
    return _cached["nc"]


def _quant_scale(a):
    m = float(np.max(np.abs(a)))
    return max(m, 1e-30) / 127.0


def _in_maps(x, attn, D, alpha):
    a0 = float(np.asarray(alpha).reshape(-1)[0])

    x = np.asarray(x, dtype=np.float32)
    attn = np.asarray(attn, dtype=np.float32)
    D = np.asarray(D, dtype=np.float32)

    s_x = _quant_scale(x)
    s_a = _quant_scale(attn)
    s_d = _quant_scale(D)
    lam = np.full((128, 1), a0 * s_a * s_d / s_x, dtype=np.float32)

    # x[n, c, p] -> xp[core, r, n, j, p] with c = 128j + r, quantized int8
    xr = np.rint(x * (1.0 / s_x)).reshape(N_CORES, NB, 4, 128, HW)
    xq = np.ascontiguousarray(xr.transpose(0, 3, 1, 2, 4)).astype(np.int8)

    # attn[n, p, q] -> attn^T[n, q, p], quantized; n = (pair, parity)
    at = np.rint(attn.transpose(0, 2, 1) * (1.0 / s_a))
    at = at.reshape(N_CORES, NPAIRS, 2, HW, HW)

    # combined cb[core, (parity, q), pair, 610]: cols 0:512 = D rows,
    # cols 512:610 = (slot, p) block-diagonal attn^T rhs: slot 0 rows 0:49
    # = A^T_even, slot 1 rows 49:98 = A^T_odd, other halves ZERO
    dr = np.rint(D * (1.0 / s_d)).reshape(N_CORES, NPAIRS, 2, HW, C)
    cb = np.zeros((N_CORES, 2 * HW, NPAIRS, CW), dtype=np.int8)
    cb[:, 0:HW, :, 0:C] = dr[:, :, 0].transpose(0, 2, 1, 3)
    cb[:, HW:2 * HW, :, 0:C] = dr[:, :, 1].transpose(0, 2, 1, 3)
    cb[:, 0:HW, :, C:C + HW] = at[:, :, 0].transpose(0, 2, 1, 3)
    cb[:, HW:2 * HW, :, C + HW:] = at[:, :, 1].transpose(0, 2, 1, 3)

    return [
        {"xp": xq[c], "cb": cb[c], "lam": lam}
        for c in range(N_CORES)
    ], s_x


def kernel(x: np.ndarray, attn: np.ndarray, D: np.ndarray, alpha: np.ndarray) -> np.ndarray:
    from concourse import bass_utils

    nc = _get_nc()
    in_maps, s_x = _in_maps(x, attn, D, alpha)
    res = bass_utils.run_bass_kernel_spmd(
        nc, in_maps, core_ids=list(range(N_CORES))
    )
    # op[r, n, j, p] -> out[n, c, p] with c = 128j + r; undo the s_x scaling
    out = np.stack([res.results[c]["op"] for c in range(N_CORES)])
    out = (out.astype(np.float32) * np.float32(s_x)).transpose(0, 2, 3, 1, 4)
    return np.ascontiguousarray(out).reshape(N_TOT, C, 7, 7)
